# revision 4
# baseline (speedup 1.0000x reference)
"""AnchorTriangleAttention on 8 Trainium2 NeuronCores via a Bass/Tile kernel.

Sharding (per spec hint): row-parallel over the first residue axis i.
Each core owns Li = L/8 = 64 rows. Host precomputes (cheap, tiny):
  - the template-gate MLP scalar, folded into the template tensors,
  - anchor gathers + the small anchor projections (left/v_left per i,
    right/v_right per j) laid out exactly as the kernel's matmuls want,
  - bf16 casts and the [i, d, j] transpose of pair_repr.
Device per core computes, for each owned row i:
  qT_i = Wq'^T xT_i                     [64a, 512j]   (Wq' = Wq/sqrt(A))
  S_i[k,j] = leftT_i^T qT_i + S2[k,j,i] - |g(t_l+t_r-t_i)|
  attn = softmax_k S_i   (exp + ones-matmul denom + reciprocal)
  U_i = v_leftT_i^T attn + U2[:,j,i]
  delta_i = sigmoid(Wg^T xT_i + bg) * (Wo^T U_i)     [128d, 512j]
where S2/U2 are the per-j "right" cross terms, computed by 512 small
matmuls per phase against strided slices of qT / attn.
Output: bf16 deltaT per core; host adds the f32 residual.

Hardcoded: B=1, L=512, K=32, D=128, A=64, SIGMA=4.0, 8 cores.
"""

import functools

import numpy as np

DIM = 128
ATTN_DIM = 64
K = 32
L = 512
B = 1
SIGMA = 4.0
N_CORES = 8
LI = L // N_CORES  # 64 rows of i per core
IB = 32            # i-block (2 blocks per core)
JT = 64            # j-tile for streaming R/VR
PACK = 4           # j's packed per PSUM bank in cross-term phases


def _template_gate_host(template_dist, template_quality, Tg_W1, Tg_b1, Tg_W2, Tg_b2):
    td = np.asarray(template_dist, dtype=np.float32)
    mask = (td > 0).astype(np.float32)
    coverage = mask.mean(axis=(1, 2))
    length = td.shape[-1]
    length_norm = np.full_like(coverage, length / 512.0)
    feats = np.stack(
        [coverage, np.asarray(template_quality, np.float32), length_norm], axis=-1
    )
    h = np.maximum(feats @ np.asarray(Tg_W1, np.float32) + np.asarray(Tg_b1, np.float32), 0.0)
    z = h @ np.asarray(Tg_W2, np.float32) + np.asarray(Tg_b2, np.float32)
    gate = 1.0 / (1.0 + np.exp(-z))
    return float(gate.reshape(-1)[0])


def _build_bass_fn():
    """Returns the bass_jit-able per-core kernel function."""
    import concourse.bass as bass
    from concourse import mybir
    from concourse.tile import TileContext

    f32 = mybir.dt.float32
    bf16 = mybir.dt.bfloat16
    fp16 = mybir.dt.float16
    AF = mybir.ActivationFunctionType
    ALU = mybir.AluOpType

    def kernel_fn(nc, xT, LT, VL, R, VR, TR, TL, TI, ONES, ONES32, WQ, WO, WG, BG):
        # Per-core DRAM shapes:
        #  xT  [64, 128, 512] bf16   pair rows, d-major
        #  LT  [2, 64, 32, 32] bf16  leftT  per block: [a, i_in_block, k]
        #  VL  [2, 32, 32, 64] bf16  v_left per block: [k, i_in_block, a]
        #  R   [8, 64, 64, 32] bf16  rightT  per j-tile: [a, j_in_tile, k]
        #  VR  [8, 32, 64, 64] bf16  v_right per j-tile: [k, j_in_tile, a]
        #  TR  [32, 512] f32         g * t_r^T     (k, j)
        #  TL  [32, 64]  f32         g * t_l^T     (k, i)
        #  TI  [64, 512] f32         g * t_i rows  (i, j)
        #  ONES [32, 32] bf16
        #  WQ  [128, 64] bf16 (pre-scaled by 1/sqrt(A)), WO [64, 128], WG [128, 128]
        #  BG  [128, 1] f32
        out = nc.dram_tensor("deltaT", [LI, DIM, L], bf16, kind="ExternalOutput")

        with TileContext(nc) as tc:
            with (
                tc.tile_pool(name="const", bufs=1) as cpool,
                tc.tile_pool(name="xin", bufs=3) as xin,
                tc.tile_pool(name="persist", bufs=1) as pers,
                tc.tile_pool(name="stream", bufs=2) as stream,
                tc.tile_pool(name="work", bufs=3) as work,
                tc.tile_pool(name="outp", bufs=3) as outp,
                tc.tile_pool(name="ps", bufs=2, space="PSUM") as ps,
            ):
                ones_sb = cpool.tile_from(ONES[:])
                ones32_sb = cpool.tile_from(ONES32[:])
                wq_sb = cpool.tile_from(WQ[:])
                wo_sb = cpool.tile_from(WO[:])
                wg_sb = cpool.tile_from(WG[:])
                bg_sb = cpool.tile_from(BG[:])
                tr_sb = cpool.tile_from(TR[:])
                tl_sb = cpool.tile_from(TL[:])

                for b in range(2):
                    lt_sb = stream.tile([ATTN_DIM, IB, K], bf16, tag="lt")
                    nc.sync.dma_start(out=lt_sb[:], in_=LT[b])
                    vl_sb = stream.tile([K, IB, ATTN_DIM], bf16, tag="vl")
                    nc.sync.dma_start(out=vl_sb[:], in_=VL[b])

                    qt_sb = pers.tile([ATTN_DIM, IB, L], bf16, tag="qt")
                    s2_sb = pers.tile([K, L, IB], fp16, tag="s2")
                    at_sb = pers.tile([K, IB, L], bf16, tag="at")
                    u2_sb = pers.tile([ATTN_DIM, L, IB], fp16, tag="u2")

                    # ---- P1: qT for the block ----
                    for ii in range(IB):
                        i = b * IB + ii
                        xt = xin.tile([DIM, L], bf16, tag="x1")
                        nc.sync.dma_start(out=xt[:], in_=xT[i])
                        qps = ps.tile([ATTN_DIM, L], f32, tag="pA")
                        nc.tensor.matmul(qps[:], wq_sb[:], xt[:], start=True, stop=True)
                        nc.scalar.activation(qt_sb[:, ii, :], qps[:], AF.Copy)

                    # ---- P2: S2[k, j, i] cross terms ----
                    for jt in range(L // JT):
                        rt = stream.tile([ATTN_DIM, JT, K], bf16, tag="rt")
                        nc.sync.dma_start(out=rt[:], in_=R[jt])
                        for jj in range(0, JT, PACK):
                            s2ps = ps.tile([K, PACK, IB], f32, tag="pA")
                            for p in range(PACK):
                                j = jt * JT + jj + p
                                nc.tensor.matmul(
                                    s2ps[:, p, :], rt[:, jj + p, :], qt_sb[:, :, j],
                                    start=True, stop=True,
                                )
                            j0 = jt * JT + jj
                            eng = nc.scalar if (jj // PACK) % 2 == 0 else nc.vector
                            if eng is nc.scalar:
                                nc.scalar.activation(
                                    s2_sb[:, j0:j0 + PACK, :], s2ps[:], AF.Copy)
                            else:
                                nc.vector.tensor_copy(
                                    s2_sb[:, j0:j0 + PACK, :], s2ps[:])

                    # ---- P3: scores + bias + softmax ----
                    for ii in range(IB):
                        i = b * IB + ii
                        ti = xin.tile([1, L], f32, tag="ti")
                        nc.sync.dma_start(out=ti[:], in_=TI[i:i + 1, :])
                        bc = ps.tile([K, L], f32, tag="pB")
                        nc.tensor.matmul(bc[:], ones32_sb[:1, :], ti[:], start=True, stop=True)
                        tmp = work.tile([K, L], f32, tag="tmp")
                        # tmp = (TR + TL[:, i]) - broadcast(TI[i])
                        nc.vector.scalar_tensor_tensor(
                            tmp[:], tr_sb[:], tl_sb[:, i:i + 1], bc[:],
                            op0=ALU.add, op1=ALU.subtract,
                        )
                        absb = work.tile([K, L], f32, tag="abs")
                        nc.scalar.activation(absb[:], tmp[:], AF.Abs)

                        sps = ps.tile([K, L], f32, tag="pC")
                        nc.tensor.matmul(
                            sps[:], lt_sb[:, ii, :], qt_sb[:, ii, :],
                            start=True, stop=True,
                        )
                        # S = S - |bias| + S2
                        nc.vector.scalar_tensor_tensor(
                            sps[:], absb[:], -1.0, sps[:],
                            op0=ALU.mult, op1=ALU.add,
                        )
                        nc.vector.tensor_tensor(
                            sps[:], sps[:], s2_sb[:, :, ii], op=ALU.add)
                        nc.scalar.activation(at_sb[:, ii, :], sps[:], AF.Exp)
                        den = ps.tile([1, L], f32, tag="pB")
                        nc.tensor.matmul(
                            den[:], ones_sb[:, :1], at_sb[:, ii, :],
                            start=True, stop=True,
                        )
                        rc = work.tile([1, L], f32, tag="rc")
                        nc.vector.reciprocal(rc[:], den[:])
                        rb = ps.tile([K, L], f32, tag="pD")
                        nc.tensor.matmul(rb[:], ones32_sb[:1, :], rc[:], start=True, stop=True)
                        nc.vector.tensor_tensor(
                            at_sb[:, ii, :], at_sb[:, ii, :], rb[:], op=ALU.mult)

                    # ---- P4: U2[a, j, i] cross terms ----
                    for jt in range(L // JT):
                        vrt = stream.tile([K, JT, ATTN_DIM], bf16, tag="vrt")
                        nc.sync.dma_start(out=vrt[:], in_=VR[jt])
                        for jj in range(0, JT, PACK):
                            u2ps = ps.tile([ATTN_DIM, PACK, IB], f32, tag="pA")
                            for p in range(PACK):
                                j = jt * JT + jj + p
                                nc.tensor.matmul(
                                    u2ps[:, p, :], vrt[:, jj + p, :], at_sb[:, :, j],
                                    start=True, stop=True,
                                )
                            j0 = jt * JT + jj
                            eng_scalar = (jj // PACK) % 2 == 1
                            if eng_scalar:
                                nc.scalar.activation(
                                    u2_sb[:, j0:j0 + PACK, :], u2ps[:], AF.Copy)
                            else:
                                nc.vector.tensor_copy(
                                    u2_sb[:, j0:j0 + PACK, :], u2ps[:])

                    # ---- P5: values, output proj, gate, delta ----
                    for ii in range(IB):
                        i = b * IB + ii
                        ups = ps.tile([ATTN_DIM, L], f32, tag="pB")
                        nc.tensor.matmul(
                            ups[:], vl_sb[:, ii, :], at_sb[:, ii, :],
                            start=True, stop=True,
                        )
                        nc.vector.tensor_tensor(
                            ups[:], ups[:], u2_sb[:, :, ii], op=ALU.add)
                        usb = work.tile([ATTN_DIM, L], bf16, tag="usb")
                        nc.scalar.activation(usb[:], ups[:], AF.Copy)
                        ops_ = ps.tile([DIM, L], f32, tag="pC")
                        nc.tensor.matmul(ops_[:], wo_sb[:], usb[:], start=True, stop=True)

                        xt2 = xin.tile([DIM, L], bf16, tag="x2")
                        nc.sync.dma_start(out=xt2[:], in_=xT[i])
                        gps = ps.tile([DIM, L], f32, tag="pD")
                        nc.tensor.matmul(gps[:], wg_sb[:], xt2[:], start=True, stop=True)
                        gsb = work.tile([DIM, L], bf16, tag="gsb")
                        nc.scalar.activation(
                            gsb[:], gps[:], AF.Sigmoid, bias=bg_sb[:, :1])
                        dsb = outp.tile([DIM, L], bf16, tag="dsb")
                        nc.vector.tensor_tensor(dsb[:], ops_[:], gsb[:], op=ALU.mult)
                        nc.sync.dma_start(out=out[i], in_=dsb[:])

        return (out,)

    return kernel_fn


@functools.lru_cache(maxsize=1)
def _get_jitted():
    import jax
    import numpy as _np
    from jax.sharding import Mesh, PartitionSpec as P
    from jax.experimental.shard_map import shard_map
    from concourse.bass2jax import bass_jit

    devices = jax.devices()[:N_CORES]
    assert len(devices) >= N_CORES
    mesh = Mesh(_np.array(devices), ("core",))
    bfn = bass_jit(_build_bass_fn())

    def body(xT, LT, VL, R, VR, TR, TL, TI, ONES, ONES32, WQ, WO, WG, BG):
        (out,) = bfn(xT, LT, VL, R, VR, TR, TL, TI, ONES, ONES32, WQ, WO, WG, BG)
        return out

    shard = P("core")
    rep = P()
    in_specs = (shard, shard, shard, rep, rep, rep, shard, shard,
                rep, rep, rep, rep, rep, rep)
    jitted = jax.jit(shard_map(
        body, mesh=mesh, in_specs=in_specs, out_specs=shard, check_rep=False))
    row = jax.sharding.NamedSharding(mesh, shard)
    repl = jax.sharding.NamedSharding(mesh, rep)
    return jitted, row, repl


def kernel(
    pair_repr, template_dist, template_quality,
    Wq, Wl, Wr, Wvl, Wvr, Wo, Wg, bg,
    Tg_W1, Tg_b1, Tg_W2, Tg_b2, anchor_idx,
):
    import jax
    import ml_dtypes

    bf16 = ml_dtypes.bfloat16
    f32 = np.float32

    pr = np.asarray(pair_repr, f32)[0]          # [L, L, D]
    td = np.asarray(template_dist, f32)[0]      # [L, L]
    aidx = np.asarray(anchor_idx).astype(np.int64)

    gate = _template_gate_host(
        np.asarray(template_dist, f32), np.asarray(template_quality, f32),
        Tg_W1, Tg_b1, Tg_W2, Tg_b2)
    g = np.float32(gate / SIGMA)

    prb = pr.astype(bf16)
    xT = np.ascontiguousarray(prb.transpose(0, 2, 1))          # [L, D, L]

    xa = pr[:, aidx, :]                                        # [L, K, D]
    xr = pr[aidx, :, :]                                        # [K, L, D]

    Wl32 = np.asarray(Wl, f32); Wvl32 = np.asarray(Wvl, f32)
    Wr32 = np.asarray(Wr, f32); Wvr32 = np.asarray(Wvr, f32)

    left = xa.reshape(-1, DIM) @ Wl32                          # [L*K, A]
    v_left = xa.reshape(-1, DIM) @ Wvl32
    left = left.reshape(L, K, ATTN_DIM)
    v_left = v_left.reshape(L, K, ATTN_DIM)
    right = (xr.reshape(-1, DIM) @ Wr32).reshape(K, L, ATTN_DIM)
    v_right = (xr.reshape(-1, DIM) @ Wvr32).reshape(K, L, ATTN_DIM)

    # LT: [cores*2, A, IB, K]  leftT per i-block
    LT = np.ascontiguousarray(
        left.reshape(L // IB, IB, K, ATTN_DIM).transpose(0, 3, 1, 2)
    ).astype(bf16)
    # VL: [cores*2, K, IB, A]
    VL = np.ascontiguousarray(
        v_left.reshape(L // IB, IB, K, ATTN_DIM).transpose(0, 2, 1, 3)
    ).astype(bf16)
    # R: [L/JT, A, JT, K]  rightT per j-tile (j-sharded for all_gather)
    R = np.ascontiguousarray(
        right.reshape(K, L // JT, JT, ATTN_DIM).transpose(1, 3, 2, 0)
    ).astype(bf16)
    # VR: [L/JT, K, JT, A]
    VR = np.ascontiguousarray(
        v_right.reshape(K, L // JT, JT, ATTN_DIM).transpose(1, 0, 2, 3)
    ).astype(bf16)

    TR = np.ascontiguousarray(td[aidx, :] * g)                 # [K, L] f32
    # TL: per-core [K, LI] stacked on axis 0 -> [cores*K, LI]
    tl = (td[:, aidx] * g).T                                   # [K, L]
    TL = np.ascontiguousarray(
        tl.reshape(K, N_CORES, LI).transpose(1, 0, 2).reshape(N_CORES * K, LI))
    TI = np.ascontiguousarray(td * g)                          # [L, L] f32

    ONES = np.ones((K, K), dtype=bf16)
    ONES32 = np.ones((K, K), dtype=f32)
    WQ = (np.asarray(Wq, f32) / np.sqrt(np.float32(ATTN_DIM))).astype(bf16)
    WOc = np.asarray(Wo, f32).astype(bf16)
    WGc = np.asarray(Wg, f32).astype(bf16)
    BG = np.asarray(bg, f32).reshape(DIM, 1)

    jitted, row, repl = _get_jitted()

    args = (
        jax.device_put(xT, row),
        jax.device_put(LT, row),
        jax.device_put(VL, row),
        jax.device_put(R, repl),
        jax.device_put(VR, repl),
        jax.device_put(TR, repl),
        jax.device_put(TL, row),
        jax.device_put(TI, row),
        jax.device_put(ONES, repl),
        jax.device_put(ONES32, repl),
        jax.device_put(WQ, repl),
        jax.device_put(WOc, repl),
        jax.device_put(WGc, repl),
        jax.device_put(BG, repl),
    )
    deltaT = np.asarray(jitted(*args))                         # [L, D, L] bf16
    delta = deltaT.transpose(0, 2, 1).astype(f32)              # [L, L, D]
    out = pr + delta
    return out[None].astype(np.float32)
